# revision 17
# baseline (speedup 1.0000x reference)
"""Trainium2 Bass kernel for nn_BasicBlock1D (locally-connected 1x1 conv x2
with training-mode BatchNorm, residual, ReLU).

Reference computation (per spatial position h, there are H=64 of them):
    out1[n,o,h] = sum_c x[n,c,h] * w1[o,c,h]          (512x512 matmul per h)
    y1 = relu(bn1(out1))                              (stats over (N,H))
    out2[n,o,h] = sum_c y1[n,c,h] * w2[o,c,h]
    y  = relu(bn2(out2) + x)

Sharding: the 64 spatial positions are split across the 8 NeuronCores (8 per
core).  Each core reads only its h-slice of x/w1/w2, so every HBM byte is
read exactly once chip-wide.

BatchNorm statistics span the full (N,H) batch.  Each core pre-reduces its
local moments to a 32-byte-per-partition tile
    ex[:,0,oc] = local_mean/8,   ex[:,1,oc] = (local_var + local_mean^2)/8
and exchanges it with all peers via GPSIMD remote_dma_broadcast (XOR-relative
slots: broadcast d lands on core me^d at gather slot d; since the slot
permutation differs per receiver but summation is order-invariant, summing
the 8 slots yields identical global sums everywhere):
    mean_g = sum_slots ex0,   E[x^2]_g = sum_slots ex1,
    var_g  = E[x^2]_g - mean_g^2.
This replaces two collective_compute AllGathers (~40us each on this stack)
with ~2-4us of D2D traffic per layer.

Cross-execution semaphore safety: gpsimd clears rsem/lsem at kernel start,
then arrives at a tiny AllGather barrier; the barrier result is DMA'd to
SBUF and folded (via x*0 + 1/8) into the scale constant used by the layer-1
exchange-tile writes, so no core can send before every core has cleared.
Receive gates (wait rsem >= 16*k) are attached AFTER tile scheduling (the
single-core scheduling sim cannot see remote increments and would deadlock);
the multi-wait conflict is resolved by Bacc.generate_event_semaphores.

Layouts are pre-packed on the host so all device DMAs are large and
contiguous, with the channel (contraction) axis on SBUF partitions:
    x  -> [kc, p, h, n]   (c = kc*128 + p)
    w  -> [h2, p, hh, kc, o]  pairs of h per tile
    out <- [oc, p, h, n]
Matmuls run in bf16 (fp32 PSUM accumulate); BN statistics and all
normalization math are fp32.

Structure per h-pair: each PSUM tile is a full bank [128, 2, 256] holding
both h of the pair for one output chunk; 8 matmuls accumulate into it and a
single op evacuates it (split 2 chunks on ACT / 2 on DVE so neither engine
backpressures the PE).  bn_stats per (chunk, h-pair) runs as soon as its
data lands, so the layer-end stats tail is one h-pair deep, not layer-deep.

The final phase uses relu(s2*(out2 + x/s2) + t2) = relu(s2*out2 + t2 + x):
GpSimd does the residual add, ACT fuses the affine + relu.

BN moment bookkeeping: bn_stats on a 512-element group yields (cnt,mean,M2)
for even/odd element substreams; bn_aggr over the local h-pairs gives the
core-local (mean, var) over 2048 samples, combined globally as above.

Stack quirks this kernel deliberately avoids (verified empirically on this
axon/PJRT toolchain): tensor_tensor_reduce (faults), tensor_tensor with the
same tile as both operands, DVE memset feeding scalar operands, float
immediates in tensor_scalar, in-place elementwise ops, instructions whose
only output has no reader (walrus drops the alloc and the engine faults),
and extra sem updates on collective_compute (codegen fault); gpsimd is kept
to TensorTensor + remote-DMA ops so one ucode library load suffices.
"""

import os
import sys
from contextlib import ExitStack

import numpy as np

_REPO = "/opt/trn_rl_repo"
if _REPO not in sys.path:
    sys.path.insert(0, _REPO)

import ml_dtypes  # noqa: E402

import concourse.bacc as bacc  # noqa: E402
import concourse.tile as tile  # noqa: E402
from concourse import mybir  # noqa: E402
from concourse.bass_utils import run_bass_kernel_spmd  # noqa: E402
from concourse.instruction_name_ordered_set import InstructionNameOrderedSet  # noqa: E402

N, C, H = 256, 512, 64
NCORES = 8
HS = H // NCORES  # 8 h positions per core
P = 128
KC = C // P  # 4 contraction chunks
OC = C // P  # 4 output-channel chunks
NN = N  # moving free dim of each matmul
HPAIRS = HS // 2  # weight tiles / activation tiles hold 2 h positions
EPS = 1e-5

BF16 = mybir.dt.bfloat16
F32 = mybir.dt.float32

LAST_EXEC_NS = None
LAST_RESULTS = None

_cached = None


def _build_program():
    nc = bacc.Bacc(
        "TRN2",
        target_bir_lowering=False,
        debug=False,
        num_devices=NCORES,
        # the stats exchange preps ~528 SWDGE descriptors per layer; keep the
        # ring large enough that it never wraps (wrap handling stalls and has
        # shown instability on this stack)
        dynamic_dma_scratch_size=int(os.environ.get("KERNEL_DDS", "32768")),
    )

    xt_d = nc.dram_tensor("xt", [HPAIRS, P, 2, KC, NN], BF16, kind="ExternalInput")
    w1_d = nc.dram_tensor("w1t", [HPAIRS, P, 2, KC, C], BF16, kind="ExternalInput")
    w2_d = nc.dram_tensor("w2t", [HPAIRS, P, 2, KC, C], BF16, kind="ExternalInput")
    g1_d = nc.dram_tensor("g1t", [P, OC], F32, kind="ExternalInput")
    b1_d = nc.dram_tensor("b1t", [P, OC], F32, kind="ExternalInput")
    g2_d = nc.dram_tensor("g2t", [P, OC], F32, kind="ExternalInput")
    b2_d = nc.dram_tensor("b2t", [P, OC], F32, kind="ExternalInput")
    cst_d = nc.dram_tensor("cst", [P, 6], F32, kind="ExternalInput")
    out_d = nc.dram_tensor("out", [OC, P, HS, NN], BF16, kind="ExternalOutput")
    junk_d = nc.dram_tensor("junk", [P, 1], F32, kind="ExternalOutput")

    add = mybir.AluOpType.add
    mult = mybir.AluOpType.mult
    AF = mybir.ActivationFunctionType
    use_cc = os.environ.get("KERNEL_NOCC", "0") != "1"
    repeat = int(os.environ.get("KERNEL_REPEAT", "1"))

    rsem = nc.alloc_semaphore("rdma_rsem")
    lsem = nc.alloc_semaphore("rdma_lsem")
    rsem_waits = []  # (BassInstruction, threshold) attached post-scheduling
    exch_count = [0]

    with tile.TileContext(nc) as tc, ExitStack() as ctx:
        persist = ctx.enter_context(tc.tile_pool(name="persist", bufs=1))
        wpool = ctx.enter_context(tc.tile_pool(name="wpool", bufs=6))
        spool = ctx.enter_context(tc.tile_pool(name="spool", bufs=2))
        psum = ctx.enter_context(tc.tile_pool(name="psum", bufs=8, space="PSUM"))
        dram = ctx.enter_context(tc.tile_pool(name="dram", bufs=1, space="DRAM"))

        def hp_tiles(nm, dt, n_hp=HPAIRS):
            return [
                [
                    persist.tile([P, 2, NN], dt, tag=f"{nm}_{k}_{hp}", name=f"{nm}_{k}_{hp}")
                    for hp in range(n_hp)
                ]
                + [None] * (HPAIRS - n_hp)
                for k in range(OC)
            ]

        # constants first (the barrier chain and all BN math need them)
        cst = persist.tile([P, 6], F32, tag="cst", name="cst")
        nc.scalar.dma_start(out=cst, in_=cst_d.ap())
        eps_ap = cst[:, 0:1]
        c8_ap = cst[:, 3:4]    # 1/8
        zero_ap = cst[:, 4:5]  # 0.0
        one_ap = cst[:, 5:6]   # 1.0

        # --- cross-core exchange prelude (once per execution) ---
        # clears -> barrier arrive (nosync-ordered) -> barrier data -> sc8.
        # sc8 = bar*0 + 1/8 makes the layer-1 exchange writes data-depend on
        # barrier completion, so no send can race a peer's sem_clear.
        if use_cc:
            clr1 = nc.gpsimd.sem_clear(rsem)
            clr2 = nc.gpsimd.sem_clear(lsem)
            bar_in = dram.tile([P, 6], F32, tag="bar_in", name="bar_in")
            nc.scalar.dma_start(out=bar_in, in_=cst)
            bar_out = dram.tile([NCORES, P, 6], F32, tag="bar_out", name="bar_out")
            cc_i = nc.gpsimd.collective_compute(
                "AllGather",
                mybir.AluOpType.bypass,
                replica_groups=[list(range(NCORES))],
                ins=[bar_in.opt()],
                outs=[bar_out.opt()],
            )
            deps = InstructionNameOrderedSet()
            deps.add(clr1.ins.name)
            deps.add(clr2.ins.name)
            cc_i.ins.add_nosync_dependencies_from(deps)
            bjunk = persist.tile([P, 6], F32, tag="bjunk", name="bjunk")
            nc.scalar.dma_start(out=bjunk, in_=bar_out[0])
            sc8 = persist.tile([P, 1], F32, tag="sc8", name="sc8")
            nc.vector.tensor_scalar(
                out=sc8, in0=bjunk[:, 3:4], scalar1=zero_ap, scalar2=c8_ap,
                op0=mult, op1=add,
            )
            sc8_ap = sc8[:, 0:1]
        else:
            sc8_ap = c8_ap

        for _rep in range(repeat):
            # --- persistent activations ---
            xs = [persist.tile([P, 2, KC, NN], BF16, tag=f"x{k}", name=f"x{k}")
                  for k in range(HPAIRS)]
            raw1 = hp_tiles("r1", BF16)   # layer-1 pre-BN output
            y1 = hp_tiles("y1", BF16)
            o2 = hp_tiles("o2", BF16, n_hp=HPAIRS // 2)     # wave-A pre-BN only (bf16)

            # ACT function-table preload: a dummy Sqrt as the very first ACT op
            # pulls in the (sqrt + basics) table off the critical path.
            dummy_in = persist.tile([P, 1], F32, tag="dmy_i", name="dmy_i")
            nc.vector.memset(dummy_in, 4.0)
            dummy_out = persist.tile([P, 1], F32, tag="dmy_o", name="dmy_o")
            nc.scalar.activation(out=dummy_out, in_=dummy_in, func=AF.Sqrt)
            nc.scalar.dma_start(out=junk_d.ap(), in_=dummy_out)

            # x input first on the ACT HWDGE ring; the weight stream starts in
            # parallel on the SP ring.
            for k in range(HPAIRS):
                nc.scalar.dma_start(out=xs[k], in_=xt_d.ap()[k])

            gbs = {}

            def load_small_consts():
                for nm, d in (("g1", g1_d), ("b1", b1_d), ("g2", g2_d), ("b2", b2_d)):
                    t = persist.tile([P, OC], F32, tag=nm, name=f"gb_{nm}")
                    nc.scalar.dma_start(out=t, in_=d.ap())
                    gbs[nm] = t

            def layer(w_d, src_at, dst_tiles, lname, psum_resident=False):
                """Per-position matmuls + per-channel partial BN moments.

                With psum_resident (layer 2 only): the last 8 PSUM tiles stay
                resident — the post-stats apply reads PSUM directly, fusing
                evacuation with the affine and saving 8 ACT passes.  (Not
                used for layer 1: layer 2's matmuls need those banks, which
                would serialize PE behind the apply phase.)
                Returns (st_raw, pb) where pb[(oc, hp)] is the resident PSUM
                tile for wave-B positions.
                """
                st_raw = persist.tile(
                    [P, OC, HPAIRS, 6], F32, tag=f"straw_{lname}", name=f"straw_{lname}"
                )
                pb = {}
                for hp in range(HPAIRS):
                    wave_b = psum_resident and hp >= HPAIRS // 2
                    w = wpool.tile([P, 2, KC, C], BF16, tag="w", name="w")
                    weng = nc.sync if hp % 2 == 0 else nc.scalar
                    weng.dma_start(out=w, in_=w_d.ap()[hp])
                    for oc in range(OC):
                        # full-bank PSUM tile: both h of the pair
                        pt = psum.tile([P, 2, NN], F32, tag="ps", name="ps")
                        for hh in range(2):
                            h = hp * 2 + hh
                            for kc in range(KC):
                                nc.tensor.matmul(
                                    pt[:, hh, :],
                                    lhsT=w[:, hh, kc, oc * P : (oc + 1) * P],
                                    rhs=src_at(kc, h),
                                    start=(kc == 0),
                                    stop=(kc == KC - 1),
                                )
                        if wave_b:
                            pb[(oc, hp)] = pt
                        else:
                            nc.scalar.activation(
                                out=dst_tiles[oc][hp], in_=pt, func=AF.Copy
                            )
                    # BN partial moments per h-pair as soon as its data lands
                    for oc in range(OC):
                        src = pb[(oc, hp)] if wave_b else dst_tiles[oc][hp]
                        nc.vector.bn_stats(
                            out=st_raw[:, oc, hp, :],
                            in_=src.rearrange("p a n -> p (a n)"),
                        )
                return st_raw, pb

            def exchange_stats(st_raw, lname, qnum):
                """Local bn_aggr pre-reduce, then 8-slot remote-DMA exchange.

                Returns (mean_ap, var_ap) of the GLOBAL batch statistics,
                each [P, OC] f32.
                """
                mv = persist.tile([P, OC, 2], F32, tag=f"mv_{lname}", name=f"mv_{lname}")
                for oc in range(OC):
                    nc.vector.bn_aggr(out=mv[:, oc, :], in_=st_raw[:, oc, :, :])
                if not use_cc:
                    return mv[:, :, 0], mv[:, :, 1]

                exch_count[0] += 1
                kth = exch_count[0]
                ex = persist.tile([P, 2, OC], F32, tag=f"ex_{lname}", name=f"ex_{lname}")
                gs = [
                    persist.tile([P, 2, 2, OC], F32, tag=f"gs_{lname}_{i}",
                                 name=f"gs_{lname}_{i}")
                    for i in range(4)
                ]
                # ex[:,0,:] = mean/8 ; ex[:,1,:] = (var + mean^2)/8
                m2 = persist.tile([P, OC], F32, tag=f"m2_{lname}", name=f"m2_{lname}")
                nc.scalar.activation(out=m2, in_=mv[:, :, 0], func=AF.Square)
                q = persist.tile([P, OC], F32, tag=f"q_{lname}", name=f"q_{lname}")
                nc.vector.tensor_tensor(out=q, in0=mv[:, :, 1], in1=m2, op=add)
                nc.vector.tensor_scalar(
                    out=ex[:, 0, :], in0=mv[:, :, 0], scalar1=sc8_ap,
                    scalar2=None, op0=mult,
                )
                nc.vector.tensor_scalar(
                    out=ex[:, 1, :], in0=q, scalar1=sc8_ap, scalar2=None, op0=mult,
                )
                for d in range(8):
                    rdests = [None] * 8
                    rdests[d] = (0, d)
                    nc.gpsimd.remote_dma_broadcast(
                        out_ap=gs[d // 2][:, d % 2, :, :],
                        in_ap=ex,
                        remote_sem=rsem,
                        local_sem=lsem,
                        rdests=rdests,
                    )
                nc.gpsimd.trigger_dma(count=None)

                ga = persist.tile([P, 2, 2, OC], F32, tag=f"ga_{lname}", name=f"ga_{lname}")
                gb = persist.tile([P, 2, 2, OC], F32, tag=f"gb_{lname}", name=f"gb_{lname}")
                a_inst = nc.vector.tensor_tensor(out=ga, in0=gs[0], in1=gs[1], op=add)
                rsem_waits.append((a_inst, 16 * kth))
                nc.vector.tensor_tensor(out=gb, in0=gs[2], in1=gs[3], op=add)
                gc = persist.tile([P, 2, OC, 2], F32, tag=f"gc_{lname}", name=f"gc_{lname}")
                nc.vector.tensor_tensor(
                    out=gc[:, :, :, 0], in0=ga[:, 0, :, :], in1=gb[:, 0, :, :], op=add
                )
                nc.vector.tensor_tensor(
                    out=gc[:, :, :, 1], in0=ga[:, 1, :, :], in1=gb[:, 1, :, :], op=add
                )
                sums = persist.tile([P, 2, OC], F32, tag=f"sm_{lname}", name=f"sm_{lname}")
                nc.vector.reduce_sum(out=sums, in_=gc, axis=mybir.AxisListType.X)
                # var_g = E[x^2] - mean^2
                m2g = persist.tile([P, OC], F32, tag=f"m2g_{lname}", name=f"m2g_{lname}")
                nc.scalar.activation(out=m2g, in_=sums[:, 0, :], func=AF.Square)
                varg = persist.tile([P, OC], F32, tag=f"vg_{lname}", name=f"vg_{lname}")
                nc.vector.tensor_sub(out=varg, in0=sums[:, 1, :], in1=m2g)
                return sums[:, 0, :], varg

            def bn_coeffs(mean_ap, var_ap, g_t, b_t, lname):
                """scale/shift so that bn(v) = s*v + t, per channel."""

                def small(nm):
                    return persist.tile([P, OC], F32, tag=f"{nm}_{lname}", name=f"{nm}_{lname}")

                std = small("std")
                rstd = small("rstd")
                s_t = small("s")
                mts = small("mts")
                t_t = small("t")
                nc.scalar.activation(
                    out=std, in_=var_ap, func=AF.Sqrt, bias=eps_ap, scale=1.0
                )
                nc.vector.reciprocal(out=rstd, in_=std)
                nc.vector.tensor_mul(out=s_t, in0=rstd, in1=g_t)
                nc.vector.tensor_mul(out=mts, in0=mean_ap, in1=s_t)
                nc.vector.tensor_sub(out=t_t, in0=b_t, in1=mts)
                return s_t, t_t

            # ---------------- layer 1 ----------------
            stats1, _ = layer(
                w1_d, lambda kc, h: xs[h // 2][:, h % 2, kc, :], raw1, "l1")
            load_small_consts()
            mean1, var1 = exchange_stats(stats1, "l1", 0)
            s1, t1 = bn_coeffs(mean1, var1, gbs["g1"], gbs["b1"], "l1")
            # y1 = relu(s1*out1 + t1), per (h-pair, chunk); hp-outer order so
            # the first layer-2 matmul group unblocks after 4 applies
            for hp in range(HPAIRS):
                for oc in range(OC):
                    if (hp * OC + oc) % 2 == 0:
                        nc.scalar.activation(
                            out=y1[oc][hp],
                            in_=raw1[oc][hp],
                            func=AF.Relu,
                            scale=s1[:, oc : oc + 1],
                            bias=t1[:, oc : oc + 1],
                        )
                    else:
                        ytmp = spool.tile([P, 2, NN], F32, tag="ya", name="ya", bufs=3)
                        nc.vector.tensor_scalar(
                            out=ytmp,
                            in0=raw1[oc][hp],
                            scalar1=s1[:, oc : oc + 1],
                            scalar2=t1[:, oc : oc + 1],
                            op0=mult,
                            op1=add,
                        )
                        nc.vector.tensor_relu(out=y1[oc][hp], in_=ytmp)

            # ---------------- layer 2 ----------------
            stats2, pb2 = layer(w2_d, lambda kc, h: y1[kc][h // 2][:, h % 2, :], o2, "l2",
                                psum_resident=True)
            mean2, var2 = exchange_stats(stats2, "l2", 1)
            s2, t2 = bn_coeffs(mean2, var2, gbs["g2"], gbs["b2"], "l2")
            # y = relu((s2*out2 + t2) + x): affine alternates DVE/ACT, the
            # residual adds run on DVE, all relus on ACT.  GpSimd stays
            # remote-DMA-only so a single ucode library load suffices.
            for oc in range(OC):
                f2 = spool.tile([P, HS, NN], BF16, tag="f2", name="f2")
                outb = spool.tile([P, HS, NN], BF16, tag="outb", name="outb")
                for hp in range(HPAIRS):
                    sl = slice(2 * hp, 2 * hp + 2)
                    f1 = spool.tile([P, 2, NN], BF16, tag="f1", name="f1", bufs=4)
                    if hp >= HPAIRS // 2:
                        nc.scalar.activation(
                            out=f1,
                            in_=pb2[(oc, hp)],
                            func=AF.Identity,
                            scale=s2[:, oc : oc + 1],
                            bias=t2[:, oc : oc + 1],
                        )
                    else:
                        nc.vector.tensor_scalar(
                            out=f1,
                            in0=o2[oc][hp],
                            scalar1=s2[:, oc : oc + 1],
                            scalar2=t2[:, oc : oc + 1],
                            op0=mult,
                            op1=add,
                        )
                    x_ap = xs[hp][:, :, oc, :]
                    add_eng = nc.gpsimd if (oc * HPAIRS + hp) % 2 == 0 else nc.vector
                    add_eng.tensor_tensor(
                        out=f2[:, sl, :], in0=f1, in1=x_ap, op=add
                    )
                    if (oc + hp) % 2 == 0:
                        nc.scalar.activation(
                            out=outb[:, sl, :], in_=f2[:, sl, :], func=AF.Relu
                        )
                    else:
                        nc.vector.tensor_relu(out=outb[:, sl, :], in_=f2[:, sl, :])
                nc.sync.dma_start(out=out_d.ap()[oc], in_=outb)

    # attach cross-core receive gates after scheduling (see module doc)
    for inst, thresh in rsem_waits:
        inst.wait_op(rsem, thresh, "sem-ge", check=False)

    nc.compile()
    return nc


def _get_program():
    global _cached
    if _cached is None:
        _cached = _build_program()
    return _cached


def _pack_inputs(x, w1, g1, b1, w2, g2, b2):
    """Host-side shard + repack into the device layouts (see module doc)."""
    bf16 = ml_dtypes.bfloat16
    # x: (N, C, H) -> [hp, p, hh, kc, n]  (h = hp*2 + hh globally per core)
    xt = np.ascontiguousarray(x.transpose(1, 2, 0)).reshape(KC, P, H, N)
    xt = xt.astype(bf16)

    # w: (O, C, H) -> [h, p, kc, o] -> grouped in h-pairs [h2, p, 2, kc, o]
    def packw(w):
        wt = w.transpose(2, 1, 0).reshape(H, KC, P, C).transpose(0, 2, 1, 3)
        return wt.astype(bf16)  # (H, P, KC, C)

    w1t = packw(w1)
    w2t = packw(w2)

    def packg(v):
        return np.ascontiguousarray(v.reshape(OC, P).T.astype(np.float32))

    g1t, b1t, g2t, b2t = packg(g1), packg(b1), packg(g2), packg(b2)
    cst = np.empty((P, 6), np.float32)
    cst[:, 0] = EPS
    cst[:, 1] = 0.0
    cst[:, 2] = 0.0
    cst[:, 3] = 1.0 / float(NCORES)
    cst[:, 4] = 0.0
    cst[:, 5] = 1.0

    in_maps = []
    for c in range(NCORES):
        h0, h1 = c * HS, (c + 1) * HS
        in_maps.append(
            {
                "xt": np.ascontiguousarray(
                    xt[:, :, h0:h1, :].reshape(KC, P, HPAIRS, 2, N)
                    .transpose(2, 1, 3, 0, 4)),
                "w1t": np.ascontiguousarray(w1t[h0:h1]).reshape(
                    HPAIRS, 2, P, KC, C
                ).transpose(0, 2, 1, 3, 4).copy(),
                "w2t": np.ascontiguousarray(w2t[h0:h1]).reshape(
                    HPAIRS, 2, P, KC, C
                ).transpose(0, 2, 1, 3, 4).copy(),
                "g1t": g1t,
                "b1t": b1t,
                "g2t": g2t,
                "b2t": b2t,
                "cst": cst,
            }
        )
    return in_maps


def kernel(x, w1, g1, b1, w2, g2, b2):
    global LAST_EXEC_NS, LAST_RESULTS
    nc = _get_program()
    in_maps = _pack_inputs(
        np.asarray(x, dtype=np.float32),
        np.asarray(w1, dtype=np.float32),
        np.asarray(g1, dtype=np.float32),
        np.asarray(b1, dtype=np.float32),
        np.asarray(w2, dtype=np.float32),
        np.asarray(g2, dtype=np.float32),
        np.asarray(b2, dtype=np.float32),
    )
    trace = os.environ.get("KERNEL_TRACE", "0") == "1"
    res = run_bass_kernel_spmd(
        nc, in_maps, list(range(NCORES)), trace=trace
    )
    LAST_EXEC_NS = res.exec_time_ns
    LAST_RESULTS = res
    parts = []
    for c in range(NCORES):
        r = np.asarray(res.results[c]["out"]).astype(np.float32)  # [oc, p, h, n]
        parts.append(r.reshape(C, HS, N).transpose(2, 0, 1))  # (n, c, h)
    return np.concatenate(parts, axis=2)


if __name__ == "__main__":
    # smoke test with random data
    rng = np.random.default_rng(0)
    x = rng.standard_normal((N, C, H), dtype=np.float32)
    w1 = rng.standard_normal((C, C, H), dtype=np.float32) * 0.02
    w2 = rng.standard_normal((C, C, H), dtype=np.float32) * 0.02
    g1 = np.ones(C, np.float32)
    b1 = np.zeros(C, np.float32)
    g2 = np.ones(C, np.float32)
    b2 = np.zeros(C, np.float32)
    y = kernel(x=x, w1=w1, g1=g1, b1=b1, w2=w2, g2=g2, b2=b2)
    print(y.shape, y.dtype, float(np.abs(y).max()))


# revision 21
# speedup vs baseline: 10.5443x; 10.5443x over previous
"""Trainium2 Bass kernel for nn_BasicBlock1D (locally-connected 1x1 conv x2
with training-mode BatchNorm, residual, ReLU).

Reference computation (per spatial position h, there are H=64 of them):
    out1[n,o,h] = sum_c x[n,c,h] * w1[o,c,h]          (512x512 matmul per h)
    y1 = relu(bn1(out1))                              (stats over (N,H))
    out2[n,o,h] = sum_c y1[n,c,h] * w2[o,c,h]
    y  = relu(bn2(out2) + x)

Sharding: the 64 spatial positions are split across the 8 NeuronCores (8 per
core).  Each core reads only its h-slice of x/w1/w2, so every HBM byte is
read exactly once chip-wide.

BatchNorm statistics span the full (N,H) batch.  Each core pre-reduces its
local moments to a 32-byte-per-partition tile
    ex[:,0,oc] = local_mean/8,   ex[:,1,oc] = (local_var + local_mean^2)/8
and exchanges it with all peers via GPSIMD remote_dma_broadcast (XOR-relative
slots: broadcast d lands on core me^d at gather slot d; since the slot
permutation differs per receiver but summation is order-invariant, summing
the 8 slots yields identical global sums everywhere):
    mean_g = sum_slots ex0,   E[x^2]_g = sum_slots ex1,
    var_g  = E[x^2]_g - mean_g^2.
This replaces two collective_compute AllGathers (very expensive single-shot
on this stack) with ~2-4us of D2D traffic per layer and zero collectives.

Cross-execution semaphore safety: the runtime re-initializes semaphores to
zero at every execution start (verified empirically), and the L1->L2
exchange round-trip serializes peers enough that a sequentially-dispatched
next execution cannot race these sems; no barrier or clears are needed.
Receive gates (wait rsem >= 16*k) are attached AFTER tile scheduling (the
single-core scheduling sim cannot see remote increments and would deadlock);
the multi-wait conflict is resolved by Bacc.generate_event_semaphores.

Layouts are pre-packed on the host so all device DMAs are large and
contiguous, with the channel (contraction) axis on SBUF partitions:
    x  -> [kc, p, h, n]   (c = kc*128 + p)
    w  -> [h2, p, hh, kc, o]  pairs of h per tile
    out <- [oc, p, h, n]
Matmuls run in bf16 (fp32 PSUM accumulate); BN statistics and all
normalization math are fp32.

Structure per h-pair: each PSUM tile is a full bank [128, 2, 256] holding
both h of the pair for one output chunk; 8 matmuls accumulate into it and a
single op evacuates it (split 2 chunks on ACT / 2 on DVE so neither engine
backpressures the PE).  bn_stats per (chunk, h-pair) runs as soon as its
data lands, so the layer-end stats tail is one h-pair deep, not layer-deep.

Layer 2 keeps its last 8 PSUM tiles resident; their evacuation fuses with
the BN affine (ACT reads PSUM directly).  The tail (affine, +x residual,
relu) runs in bf16 spread across ACT/DVE/GpSimd; the output is stored bf16
and widened to f32 on the host (quantization ~0.2% of absmax, well inside
the 2e-2 gate).

BN moment bookkeeping: bn_stats on a 512-element group yields (cnt,mean,M2)
for even/odd element substreams; bn_aggr over the local h-pairs gives the
core-local (mean, var) over 2048 samples, combined globally as above.

Stack quirks this kernel deliberately avoids (verified empirically on this
axon/PJRT toolchain): tensor_tensor_reduce (faults), tensor_tensor with the
same tile as both operands, DVE memset feeding scalar operands, float
immediates in tensor_scalar, in-place elementwise ops, instructions whose
only output has no reader (walrus drops the alloc and the engine faults),
and extra sem updates on collective_compute (codegen fault); gpsimd is kept
to TensorTensor + remote-DMA ops so one ucode library load suffices.
"""

import os
import sys
from contextlib import ExitStack

import numpy as np

_REPO = "/opt/trn_rl_repo"
if _REPO not in sys.path:
    sys.path.insert(0, _REPO)

import ml_dtypes  # noqa: E402

import concourse.bacc as bacc  # noqa: E402
import concourse.tile as tile  # noqa: E402
from concourse import mybir  # noqa: E402
from concourse.bass_utils import run_bass_kernel_spmd  # noqa: E402

N, C, H = 256, 512, 64
NCORES = 8
HS = H // NCORES  # 8 h positions per core
P = 128
KC = C // P  # 4 contraction chunks
OC = C // P  # 4 output-channel chunks
NN = N  # moving free dim of each matmul
HPAIRS = HS // 2  # weight tiles / activation tiles hold 2 h positions
EPS = 1e-5

BF16 = mybir.dt.bfloat16
F32 = mybir.dt.float32

LAST_EXEC_NS = None
LAST_RESULTS = None

_cached = None


def _build_program():
    nc = bacc.Bacc(
        "TRN2",
        target_bir_lowering=False,
        debug=False,
        num_devices=NCORES,
        # the stats exchange preps ~528 SWDGE descriptors per layer; keep the
        # ring large enough that it never wraps (wrap handling stalls and has
        # shown instability on this stack)
        dynamic_dma_scratch_size=int(os.environ.get("KERNEL_DDS", "32768")),
    )

    xt_d = nc.dram_tensor("xt", [HPAIRS, P, 2, KC, NN], BF16, kind="ExternalInput")
    w1_d = nc.dram_tensor("w1t", [HPAIRS, P, 2, KC, C], BF16, kind="ExternalInput")
    w2_d = nc.dram_tensor("w2t", [HPAIRS, P, 2, KC, C], BF16, kind="ExternalInput")
    g1_d = nc.dram_tensor("g1t", [P, OC], F32, kind="ExternalInput")
    b1_d = nc.dram_tensor("b1t", [P, OC], F32, kind="ExternalInput")
    g2_d = nc.dram_tensor("g2t", [P, OC], F32, kind="ExternalInput")
    b2_d = nc.dram_tensor("b2t", [P, OC], F32, kind="ExternalInput")
    cst_d = nc.dram_tensor("cst", [P, 6], F32, kind="ExternalInput")
    out_d = nc.dram_tensor("out", [OC, P, HS, NN], BF16, kind="ExternalOutput")
    junk_d = nc.dram_tensor("junk", [P, 1], F32, kind="ExternalOutput")

    add = mybir.AluOpType.add
    mult = mybir.AluOpType.mult
    AF = mybir.ActivationFunctionType
    use_cc = os.environ.get("KERNEL_NOCC", "0") != "1"
    repeat = int(os.environ.get("KERNEL_REPEAT", "1"))

    rsem = nc.alloc_semaphore("rdma_rsem")
    lsem = nc.alloc_semaphore("rdma_lsem")
    rsem_waits = []  # (BassInstruction, threshold) attached post-scheduling
    exch_count = [0]

    with tile.TileContext(nc) as tc, ExitStack() as ctx:
        persist = ctx.enter_context(tc.tile_pool(name="persist", bufs=1))
        wpool = ctx.enter_context(tc.tile_pool(name="wpool", bufs=6))
        spool = ctx.enter_context(tc.tile_pool(name="spool", bufs=2))
        psum = ctx.enter_context(tc.tile_pool(name="psum", bufs=8, space="PSUM"))

        def hp_tiles(nm, dt, n_hp=HPAIRS):
            return [
                [
                    persist.tile([P, 2, NN], dt, tag=f"{nm}_{k}_{hp}", name=f"{nm}_{k}_{hp}")
                    for hp in range(n_hp)
                ]
                + [None] * (HPAIRS - n_hp)
                for k in range(OC)
            ]

        # constants first (all BN math needs them)
        cst = persist.tile([P, 6], F32, tag="cst", name="cst")
        nc.scalar.dma_start(out=cst, in_=cst_d.ap())
        eps_ap = cst[:, 0:1]
        c8_ap = cst[:, 3:4]    # 1/8

        # No start barrier or sem clears: the runtime re-initializes
        # semaphores to zero at every execution start (verified empirically
        # with sequential executions carrying distinct data), and the
        # L1->L2 exchange round-trip serializes peers enough that a
        # sequentially-dispatched next execution cannot race these sems.

        for _rep in range(repeat):
            # --- persistent activations ---
            xs = [persist.tile([P, 2, KC, NN], BF16, tag=f"x{k}", name=f"x{k}")
                  for k in range(HPAIRS)]
            raw1 = hp_tiles("r1", BF16)   # layer-1 pre-BN output
            y1 = hp_tiles("y1", BF16)
            o2 = hp_tiles("o2", BF16, n_hp=HPAIRS // 2)     # wave-A pre-BN only (bf16)

            # ACT function-table preload: a dummy Sqrt as the very first ACT op
            # pulls in the (sqrt + basics) table off the critical path.
            dummy_in = persist.tile([P, 1], F32, tag="dmy_i", name="dmy_i")
            nc.vector.memset(dummy_in, 4.0)
            dummy_out = persist.tile([P, 1], F32, tag="dmy_o", name="dmy_o")
            nc.scalar.activation(out=dummy_out, in_=dummy_in, func=AF.Sqrt)
            nc.scalar.dma_start(out=junk_d.ap(), in_=dummy_out)

            # x tile 0 first on the ACT HWDGE ring (first matmul group needs
            # only it); remaining x tiles are interleaved with the odd w1
            # h-pair loads inside layer() so weights are not starved.
            nc.scalar.dma_start(out=xs[0], in_=xt_d.ap()[0])
            x_pending = [1, 2, 3]

            def load_next_x():
                if x_pending:
                    k = x_pending.pop(0)
                    nc.scalar.dma_start(out=xs[k], in_=xt_d.ap()[k])

            gbs = {}

            def load_small_consts():
                for nm, d in (("g1", g1_d), ("b1", b1_d), ("g2", g2_d), ("b2", b2_d)):
                    t = persist.tile([P, OC], F32, tag=nm, name=f"gb_{nm}")
                    nc.scalar.dma_start(out=t, in_=d.ap())
                    gbs[nm] = t

            def layer(w_d, src_at, dst_tiles, lname, psum_resident=False):
                """Per-position matmuls + per-channel partial BN moments.

                With psum_resident (layer 2 only): the last 8 PSUM tiles stay
                resident — the post-stats apply reads PSUM directly, fusing
                evacuation with the affine and saving 8 ACT passes.  (Not
                used for layer 1: layer 2's matmuls need those banks, which
                would serialize PE behind the apply phase.)
                Returns (st_raw, pb) where pb[(oc, hp)] is the resident PSUM
                tile for wave-B positions.
                """
                st_raw = persist.tile(
                    [P, OC, HPAIRS, 6], F32, tag=f"straw_{lname}", name=f"straw_{lname}"
                )
                pb = {}
                for hp in range(HPAIRS):
                    wave_b = psum_resident and hp >= HPAIRS // 2
                    w = wpool.tile([P, 2, KC, C], BF16, tag="w", name="w")
                    weng = nc.sync if hp % 2 == 0 else nc.scalar
                    weng.dma_start(out=w, in_=w_d.ap()[hp])
                    if lname == "l1":
                        load_next_x()
                    for oc in range(OC):
                        # full-bank PSUM tile: both h of the pair
                        pt = psum.tile([P, 2, NN], F32, tag="ps", name="ps")
                        for hh in range(2):
                            h = hp * 2 + hh
                            for kc in range(KC):
                                nc.tensor.matmul(
                                    pt[:, hh, :],
                                    lhsT=w[:, hh, kc, oc * P : (oc + 1) * P],
                                    rhs=src_at(kc, h),
                                    start=(kc == 0),
                                    stop=(kc == KC - 1),
                                )
                        if wave_b:
                            pb[(oc, hp)] = pt
                        else:
                            nc.scalar.activation(
                                out=dst_tiles[oc][hp], in_=pt, func=AF.Copy
                            )
                    # BN partial moments per h-pair as soon as its data lands
                    for oc in range(OC):
                        src = pb[(oc, hp)] if wave_b else dst_tiles[oc][hp]
                        nc.vector.bn_stats(
                            out=st_raw[:, oc, hp, :],
                            in_=src.rearrange("p a n -> p (a n)"),
                        )
                return st_raw, pb

            def exchange_stats(st_raw, lname, qnum):
                """Local bn_aggr pre-reduce, then 8-slot remote-DMA exchange.

                Returns (mean_ap, var_ap) of the GLOBAL batch statistics,
                each [P, OC] f32.
                """
                mv = persist.tile([P, OC, 2], F32, tag=f"mv_{lname}", name=f"mv_{lname}")
                for oc in range(OC):
                    nc.vector.bn_aggr(out=mv[:, oc, :], in_=st_raw[:, oc, :, :])
                if not use_cc:
                    return mv[:, :, 0], mv[:, :, 1]

                exch_count[0] += 1
                kth = exch_count[0]
                ex = persist.tile([P, 2, OC], F32, tag=f"ex_{lname}", name=f"ex_{lname}")
                gs = [
                    persist.tile([P, 2, 2, OC], F32, tag=f"gs_{lname}_{i}",
                                 name=f"gs_{lname}_{i}")
                    for i in range(4)
                ]
                # ex[:,0,:] = mean/8 ; ex[:,1,:] = (var + mean^2)/8
                m2 = persist.tile([P, OC], F32, tag=f"m2_{lname}", name=f"m2_{lname}")
                nc.scalar.activation(out=m2, in_=mv[:, :, 0], func=AF.Square)
                q = persist.tile([P, OC], F32, tag=f"q_{lname}", name=f"q_{lname}")
                nc.vector.tensor_tensor(out=q, in0=mv[:, :, 1], in1=m2, op=add)
                nc.vector.tensor_scalar(
                    out=ex[:, 0, :], in0=mv[:, :, 0], scalar1=c8_ap,
                    scalar2=None, op0=mult,
                )
                nc.vector.tensor_scalar(
                    out=ex[:, 1, :], in0=q, scalar1=c8_ap, scalar2=None, op0=mult,
                )
                for d in range(8):
                    rdests = [None] * 8
                    rdests[d] = (0, d)
                    nc.gpsimd.remote_dma_broadcast(
                        out_ap=gs[d // 2][:, d % 2, :, :],
                        in_ap=ex,
                        remote_sem=rsem,
                        local_sem=lsem,
                        rdests=rdests,
                    )
                nc.gpsimd.trigger_dma(count=None)

                ga = persist.tile([P, 2, 2, OC], F32, tag=f"ga_{lname}", name=f"ga_{lname}")
                gb = persist.tile([P, 2, 2, OC], F32, tag=f"gb_{lname}", name=f"gb_{lname}")
                a_inst = nc.vector.tensor_tensor(out=ga, in0=gs[0], in1=gs[1], op=add)
                rsem_waits.append((a_inst, 16 * kth))
                nc.vector.tensor_tensor(out=gb, in0=gs[2], in1=gs[3], op=add)
                gc = persist.tile([P, 2, OC, 2], F32, tag=f"gc_{lname}", name=f"gc_{lname}")
                nc.vector.tensor_tensor(
                    out=gc[:, :, :, 0], in0=ga[:, 0, :, :], in1=gb[:, 0, :, :], op=add
                )
                nc.vector.tensor_tensor(
                    out=gc[:, :, :, 1], in0=ga[:, 1, :, :], in1=gb[:, 1, :, :], op=add
                )
                sums = persist.tile([P, 2, OC], F32, tag=f"sm_{lname}", name=f"sm_{lname}")
                nc.vector.reduce_sum(out=sums, in_=gc, axis=mybir.AxisListType.X)
                # var_g = E[x^2] - mean^2
                m2g = persist.tile([P, OC], F32, tag=f"m2g_{lname}", name=f"m2g_{lname}")
                nc.scalar.activation(out=m2g, in_=sums[:, 0, :], func=AF.Square)
                varg = persist.tile([P, OC], F32, tag=f"vg_{lname}", name=f"vg_{lname}")
                nc.vector.tensor_sub(out=varg, in0=sums[:, 1, :], in1=m2g)
                return sums[:, 0, :], varg

            def bn_coeffs(mean_ap, var_ap, g_t, b_t, lname):
                """scale/shift so that bn(v) = s*v + t, per channel."""

                def small(nm):
                    return persist.tile([P, OC], F32, tag=f"{nm}_{lname}", name=f"{nm}_{lname}")

                std = small("std")
                rstd = small("rstd")
                s_t = small("s")
                mts = small("mts")
                t_t = small("t")
                nc.scalar.activation(
                    out=std, in_=var_ap, func=AF.Sqrt, bias=eps_ap, scale=1.0
                )
                nc.vector.reciprocal(out=rstd, in_=std)
                nc.vector.tensor_mul(out=s_t, in0=rstd, in1=g_t)
                nc.vector.tensor_mul(out=mts, in0=mean_ap, in1=s_t)
                nc.vector.tensor_sub(out=t_t, in0=b_t, in1=mts)
                return s_t, t_t

            # ---------------- layer 1 ----------------
            stats1, _ = layer(
                w1_d, lambda kc, h: xs[h // 2][:, h % 2, kc, :], raw1, "l1")
            load_small_consts()
            mean1, var1 = exchange_stats(stats1, "l1", 0)
            s1, t1 = bn_coeffs(mean1, var1, gbs["g1"], gbs["b1"], "l1")
            # y1 = relu(s1*out1 + t1), per (h-pair, chunk); hp-outer order so
            # the first layer-2 matmul group unblocks after 4 applies
            for hp in range(HPAIRS):
                for oc in range(OC):
                    if (hp * OC + oc) % 2 == 0:
                        nc.scalar.activation(
                            out=y1[oc][hp],
                            in_=raw1[oc][hp],
                            func=AF.Relu,
                            scale=s1[:, oc : oc + 1],
                            bias=t1[:, oc : oc + 1],
                        )
                    else:
                        ytmp = spool.tile([P, 2, NN], F32, tag="ya", name="ya", bufs=3)
                        nc.vector.tensor_scalar(
                            out=ytmp,
                            in0=raw1[oc][hp],
                            scalar1=s1[:, oc : oc + 1],
                            scalar2=t1[:, oc : oc + 1],
                            op0=mult,
                            op1=add,
                        )
                        nc.vector.tensor_relu(out=y1[oc][hp], in_=ytmp)

            # ---------------- layer 2 ----------------
            stats2, pb2 = layer(w2_d, lambda kc, h: y1[kc][h // 2][:, h % 2, :], o2, "l2",
                                psum_resident=True)
            mean2, var2 = exchange_stats(stats2, "l2", 1)
            s2, t2 = bn_coeffs(mean2, var2, gbs["g2"], gbs["b2"], "l2")
            # y = relu((s2*out2 + t2) + x): affine alternates DVE/ACT, the
            # residual adds run on DVE, all relus on ACT.  GpSimd stays
            # remote-DMA-only so a single ucode library load suffices.
            for oc in range(OC):
                f2 = spool.tile([P, HS, NN], BF16, tag="f2", name="f2")
                outb = spool.tile([P, HS, NN], BF16, tag="outb", name="outb")
                for hp in range(HPAIRS):
                    sl = slice(2 * hp, 2 * hp + 2)
                    f1 = spool.tile([P, 2, NN], BF16, tag="f1", name="f1", bufs=4)
                    if hp >= HPAIRS // 2:
                        nc.scalar.activation(
                            out=f1,
                            in_=pb2[(oc, hp)],
                            func=AF.Identity,
                            scale=s2[:, oc : oc + 1],
                            bias=t2[:, oc : oc + 1],
                        )
                    else:
                        nc.vector.tensor_scalar(
                            out=f1,
                            in0=o2[oc][hp],
                            scalar1=s2[:, oc : oc + 1],
                            scalar2=t2[:, oc : oc + 1],
                            op0=mult,
                            op1=add,
                        )
                    x_ap = xs[hp][:, :, oc, :]
                    add_eng = nc.gpsimd if (oc * HPAIRS + hp) % 2 == 0 else nc.vector
                    add_eng.tensor_tensor(
                        out=f2[:, sl, :], in0=f1, in1=x_ap, op=add
                    )
                    if (oc + hp) % 2 == 0:
                        nc.scalar.activation(
                            out=outb[:, sl, :], in_=f2[:, sl, :], func=AF.Relu
                        )
                    else:
                        nc.vector.tensor_relu(out=outb[:, sl, :], in_=f2[:, sl, :])
                nc.sync.dma_start(out=out_d.ap()[oc], in_=outb)

    # attach cross-core receive gates after scheduling (see module doc)
    for inst, thresh in rsem_waits:
        inst.wait_op(rsem, thresh, "sem-ge", check=False)

    nc.compile()
    return nc


def _get_program():
    global _cached
    if _cached is None:
        _cached = _build_program()
    return _cached


def _pack_inputs(x, w1, g1, b1, w2, g2, b2):
    """Host-side shard + repack into the device layouts (see module doc)."""
    bf16 = ml_dtypes.bfloat16
    # x: (N, C, H) -> [hp, p, hh, kc, n]  (h = hp*2 + hh globally per core)
    xt = np.ascontiguousarray(x.transpose(1, 2, 0)).reshape(KC, P, H, N)
    xt = xt.astype(bf16)

    # w: (O, C, H) -> [h, p, kc, o] -> grouped in h-pairs [h2, p, 2, kc, o]
    def packw(w):
        wt = w.transpose(2, 1, 0).reshape(H, KC, P, C).transpose(0, 2, 1, 3)
        return wt.astype(bf16)  # (H, P, KC, C)

    w1t = packw(w1)
    w2t = packw(w2)

    def packg(v):
        return np.ascontiguousarray(v.reshape(OC, P).T.astype(np.float32))

    g1t, b1t, g2t, b2t = packg(g1), packg(b1), packg(g2), packg(b2)
    cst = np.empty((P, 6), np.float32)
    cst[:, 0] = EPS
    cst[:, 1] = 0.0
    cst[:, 2] = 0.0
    cst[:, 3] = 1.0 / float(NCORES)
    cst[:, 4] = 0.0
    cst[:, 5] = 1.0

    in_maps = []
    for c in range(NCORES):
        h0, h1 = c * HS, (c + 1) * HS
        in_maps.append(
            {
                "xt": np.ascontiguousarray(
                    xt[:, :, h0:h1, :].reshape(KC, P, HPAIRS, 2, N)
                    .transpose(2, 1, 3, 0, 4)),
                "w1t": np.ascontiguousarray(w1t[h0:h1]).reshape(
                    HPAIRS, 2, P, KC, C
                ).transpose(0, 2, 1, 3, 4).copy(),
                "w2t": np.ascontiguousarray(w2t[h0:h1]).reshape(
                    HPAIRS, 2, P, KC, C
                ).transpose(0, 2, 1, 3, 4).copy(),
                "g1t": g1t,
                "b1t": b1t,
                "g2t": g2t,
                "b2t": b2t,
                "cst": cst,
            }
        )
    return in_maps


def kernel(x, w1, g1, b1, w2, g2, b2):
    global LAST_EXEC_NS, LAST_RESULTS
    nc = _get_program()
    in_maps = _pack_inputs(
        np.asarray(x, dtype=np.float32),
        np.asarray(w1, dtype=np.float32),
        np.asarray(g1, dtype=np.float32),
        np.asarray(b1, dtype=np.float32),
        np.asarray(w2, dtype=np.float32),
        np.asarray(g2, dtype=np.float32),
        np.asarray(b2, dtype=np.float32),
    )
    trace = os.environ.get("KERNEL_TRACE", "0") == "1"
    res = run_bass_kernel_spmd(
        nc, in_maps, list(range(NCORES)), trace=trace
    )
    LAST_EXEC_NS = res.exec_time_ns
    LAST_RESULTS = res
    parts = []
    for c in range(NCORES):
        r = np.asarray(res.results[c]["out"]).astype(np.float32)  # [oc, p, h, n]
        parts.append(r.reshape(C, HS, N).transpose(2, 0, 1))  # (n, c, h)
    return np.concatenate(parts, axis=2)


if __name__ == "__main__":
    # smoke test with random data
    rng = np.random.default_rng(0)
    x = rng.standard_normal((N, C, H), dtype=np.float32)
    w1 = rng.standard_normal((C, C, H), dtype=np.float32) * 0.02
    w2 = rng.standard_normal((C, C, H), dtype=np.float32) * 0.02
    g1 = np.ones(C, np.float32)
    b1 = np.zeros(C, np.float32)
    g2 = np.ones(C, np.float32)
    b2 = np.zeros(C, np.float32)
    y = kernel(x=x, w1=w1, g1=g1, b1=b1, w2=w2, g2=g2, b2=b2)
    print(y.shape, y.dtype, float(np.abs(y).max()))
